# revision 1
# baseline (speedup 1.0000x reference)
"""Cross-covariance (XCA / channel) attention kernel for Trainium2, 8 NeuronCores.

Reference computation (per batch b, head h, with X = x[b] in R^{N x C}):
    qkv = X @ Wqkv + bqkv;  q,k,v per head as [hd, N] (channels x tokens)
    q <- l2norm(q, axis=N) * temp_h ; k <- l2norm(k, axis=N)
    attn = softmax(q @ k^T)                # [hd, hd] channel attention
    out_h = attn @ v                       # [hd, N]
    y = concat_h(out_h)^T @ Wproj + bproj  # [N, C]

Key restructure used here (mathematically exact):
    All attention statistics derive from the per-batch Gram matrix
        S = X^T X   in R^{C x C}:
    G[h] = Wq_h^T S Wk_h          (q.k^T inner products, pre-normalization)
    ||q_d||^2 = diag(Wq_h^T S Wq_h),  ||k_e||^2 = diag(Wk_h^T S Wk_h)
    attn[h] = softmax(temp_h * G[h] / (||q|| ||k||^T))
    y = X @ M + c, where M = sum_h Wv_h @ attn[h]^T @ Wproj_h  in R^{C x C}
This reduces FLOPs from ~161 GF to ~90 GF and removes the qkv
materialization entirely (memory-friendly: two passes over X, one tiny
all-reduce of S partials between them).

Sharding: 8 cores = 4 batches x 2 sequence halves. Each core computes a
partial S over its 4096 tokens, pair-all-reduces S with its batch peer,
then computes attention/M redundantly (tiny) and produces its own 4096
output rows. Outputs are concatenated on the host.

Matmuls run in float32r (fp32 with 11-bit mantissa) at full PE rate.
"""
import numpy as np

import concourse.bacc as bacc
import concourse.mybir as mybir
import concourse.tile as tile

B, N, C = 4, 8192, 768
H, HD = 12, 64
NLOC = N // 2          # tokens per core (4096)
NCORES = 8
F32 = mybir.dt.float32
F32R = mybir.dt.float32r
AX = mybir.AxisListType.X

_CACHE = {}


def _round_fp32r(a: np.ndarray) -> np.ndarray:
    """Round fp32 to fp32r (11-bit mantissa), round-half-to-even."""
    a = np.ascontiguousarray(a, dtype=np.float32)
    v = a.view(np.uint32)
    r = (v + np.uint32(0x7FF) + ((v >> np.uint32(12)) & np.uint32(1))) & np.uint32(0xFFFFF000)
    return r.view(np.float32)


def _build(has_bias: bool, debug: bool = False):
    nc = bacc.Bacc("TRN2", target_bir_lowering=False, debug=False,
                   enable_asserts=False, num_devices=NCORES)

    # ---- per-core I/O ----
    x_d = nc.dram_tensor("x", [NLOC, C], F32R, kind="ExternalInput")
    xT_d = nc.dram_tensor("xT", [C, NLOC], F32R, kind="ExternalInput")
    wq_d = nc.dram_tensor("wq", [C, C], F32R, kind="ExternalInput")
    wk_d = nc.dram_tensor("wk", [C, C], F32R, kind="ExternalInput")
    wvt_d = nc.dram_tensor("wvt", [C, C], F32R, kind="ExternalInput")
    wproj_d = nc.dram_tensor("wproj", [C, C], F32R, kind="ExternalInput")
    tempv_d = nc.dram_tensor("tempv", [1, C], F32, kind="ExternalInput")  # temp[h] repeated per (h,d)
    ones128_d = nc.dram_tensor("ones128", [128, 1], F32R, kind="ExternalInput")
    ones1_d = nc.dram_tensor("ones1", [1, HD], F32R, kind="ExternalInput")
    if has_bias:
        gcorr_d = nc.dram_tensor("gcorr", [HD, C], F32, kind="ExternalInput")
        nq2c_d = nc.dram_tensor("nq2c", [1, C], F32, kind="ExternalInput")
        nk2c_d = nc.dram_tensor("nk2c", [1, C], F32, kind="ExternalInput")
        bvt_d = nc.dram_tensor("bvt", [128, H // 2], F32R, kind="ExternalInput")
        bproj_d = nc.dram_tensor("bproj", [1, C], F32, kind="ExternalInput")
        ones128w_d = nc.dram_tensor("ones128w", [1, 128], F32R, kind="ExternalInput")
    y_d = nc.dram_tensor("y", [NLOC, C], F32, kind="ExternalOutput")
    if debug:
        sdbg_d = nc.dram_tensor("sdbg", [C, C], F32, kind="ExternalOutput")
        gdbg_d = nc.dram_tensor("gdbg", [HD, C], F32, kind="ExternalOutput")
        adbg_d = nc.dram_tensor("adbg", [HD, C], F32, kind="ExternalOutput")
        mdbg_d = nc.dram_tensor("mdbg", [C, C], F32, kind="ExternalOutput")

    NT = NLOC // 128     # 32 token chunks of 128
    KC = C // 128        # 6 channel chunks
    FH = C // 2          # 384, free-dim half

    with tile.TileContext(nc) as tc:
        with (
            tc.tile_pool(name="big", bufs=6) as big,       # [128,6,768] 18KB/part slots
            tc.tile_pool(name="xs", bufs=4) as xs,         # x stream [128,768]
            tc.tile_pool(name="xts", bufs=3) as xts,       # xT stream [128,6,256]
            tc.tile_pool(name="yo", bufs=3) as yo,         # y out [128,768]
            tc.tile_pool(name="small", bufs=1) as small,   # small f32 tiles
            tc.tile_pool(name="const", bufs=1) as const,
            tc.tile_pool(name="ps", bufs=6, space="PSUM") as ps,
            tc.tile_pool(name="pss", bufs=2, space="PSUM") as pss,
            tc.tile_pool(name="dram", bufs=1, space="DRAM") as dram,
        ):
            # ---------------- phase 1: S_partial = x^T x ----------------
            # big slot #1
            s32_sb = big.tile([128, KC, C], F32, tag="big")
            x_view = x_d.rearrange("(t p) c -> p t c", p=128)
            for f in range(2):
                s_ps = [ps.tile([128, FH], F32, tag="ps", name=f"s_ps{f}_{i}") for i in range(KC)]
                for t in range(NT):
                    x_t = xs.tile([128, C], F32R, tag="xt")
                    nc.sync.dma_start(out=x_t[:, :], in_=x_view[:, t, :])
                    for m in range(KC):
                        nc.tensor.matmul(s_ps[m][:, :],
                                         x_t[:, m * 128:(m + 1) * 128],
                                         x_t[:, f * FH:(f + 1) * FH],
                                         start=(t == 0), stop=(t == NT - 1))
                for m in range(KC):
                    nc.vector.tensor_copy(s32_sb[:, m, f * FH:(f + 1) * FH], s_ps[m][:, :])

            # weights can load while phase 1 runs (big slots #2, #3)
            wq_sb = big.tile([128, KC, C], F32R, tag="big")
            nc.sync.dma_start(out=wq_sb[:, :, :], in_=wq_d.rearrange("(k p) c -> p k c", p=128))
            wk_sb = big.tile([128, KC, C], F32R, tag="big")
            nc.sync.dma_start(out=wk_sb[:, :, :], in_=wk_d.rearrange("(k p) c -> p k c", p=128))

            # ---------------- all-reduce S within batch pair ----------------
            s_part = dram.tile([C, C], F32)
            s_full = dram.tile([C, C], F32)
            sp_view = s_part.rearrange("(k p) c -> p k c", p=128)
            sf_view = s_full.rearrange("(k p) c -> p k c", p=128)
            nc.sync.dma_start(out=sp_view, in_=s32_sb[:, :, :])
            nc.gpsimd.collective_compute(
                "AllReduce", mybir.AluOpType.add,
                replica_groups=[[0, 1], [2, 3], [4, 5], [6, 7]],
                ins=[s_part.opt()], outs=[s_full.opt()],
            )
            nc.sync.dma_start(out=s32_sb[:, :, :], in_=sf_view)
            if debug:
                nc.sync.dma_start(out=sdbg_d.rearrange("(k p) c -> p k c", p=128),
                                  in_=s32_sb[:, :, :])

            # convert to fp32r (big slot #4)
            sr_sb = big.tile([128, KC, C], F32R, tag="big")
            nc.vector.tensor_copy(sr_sb[:, :, :], s32_sb[:, :, :])

            # ---------------- phase 2: U = S @ W, G, norms ----------------
            uq_sb = big.tile([128, KC, C], F32R, tag="big")   # slot #5
            uk_sb = big.tile([128, KC, C], F32R, tag="big")   # slot #6
            for dst, w in ((uq_sb, wq_sb), (uk_sb, wk_sb)):
                for m in range(KC):
                    for f in range(2):
                        u_ps = ps.tile([128, FH], F32, tag="ps")
                        for k in range(KC):
                            nc.tensor.matmul(u_ps[:, :],
                                             sr_sb[:, k, m * 128:(m + 1) * 128],
                                             w[:, k, f * FH:(f + 1) * FH],
                                             start=(k == 0), stop=(k == KC - 1))
                        nc.vector.tensor_copy(dst[:, m, f * FH:(f + 1) * FH], u_ps[:, :])

            # G[h] = Wq_h^T (S Wk)_h  -> [64, (h,e)]
            g_sb = small.tile([HD, C], F32, tag="g", bufs=1)
            for h in range(H):
                g_ps = pss.tile([HD, HD], F32, tag="pss")
                for k in range(KC):
                    nc.tensor.matmul(g_ps[:, :],
                                     wq_sb[:, k, h * HD:(h + 1) * HD],
                                     uk_sb[:, k, h * HD:(h + 1) * HD],
                                     start=(k == 0), stop=(k == KC - 1))
                nc.vector.tensor_copy(g_sb[:, h * HD:(h + 1) * HD], g_ps[:, :])

            ones128_sb = const.tile([128, 1], F32R, tag="ones128")
            nc.sync.dma_start(out=ones128_sb[:, :], in_=ones128_d[:, :])
            ones1_sb = const.tile([1, HD], F32R, tag="ones1")
            nc.sync.dma_start(out=ones1_sb[:, :], in_=ones1_d[:, :])
            tempv_sb = const.tile([1, C], F32, tag="tempv")
            nc.sync.dma_start(out=tempv_sb[:, :], in_=tempv_d[:, :])

            # nq2/nk2: column sums of Wq*Uq / Wk*Uk
            nrm = []
            # big slots #7 (reuses s32 slot) and #8 (reuses wq slot)
            for w, u in ((wq_sb, uq_sb), (wk_sb, uk_sb)):
                p_sb = big.tile([128, KC, C], F32R, tag="big")
                nc.vector.tensor_mul(p_sb[:, :, :], w.bitcast(F32)[:, :, :],
                                     u.bitcast(F32)[:, :, :])
                n2_sb = small.tile([1, C], F32, tag="n2", bufs=2)
                for f in range(2):
                    n_ps = pss.tile([1, FH], F32, tag="pss")
                    for k in range(KC):
                        nc.tensor.matmul(n_ps[:, :], ones128_sb[:, :],
                                         p_sb[:, k, f * FH:(f + 1) * FH],
                                         start=(k == 0), stop=(k == KC - 1))
                    nc.vector.tensor_copy(n2_sb[:, f * FH:(f + 1) * FH], n_ps[:, :])
                nrm.append(n2_sb)
            nq2_sb, nk2_sb = nrm

            if has_bias:
                nq2c_sb = const.tile([1, C], F32, tag="nq2c")
                nc.sync.dma_start(out=nq2c_sb[:, :], in_=nq2c_d[:, :])
                nk2c_sb = const.tile([1, C], F32, tag="nk2c")
                nc.sync.dma_start(out=nk2c_sb[:, :], in_=nk2c_d[:, :])
                gcorr_sb = const.tile([HD, C], F32, tag="gcorr")
                nc.sync.dma_start(out=gcorr_sb[:, :], in_=gcorr_d[:, :])
                nc.vector.tensor_add(nq2_sb[:, :], nq2_sb[:, :], nq2c_sb[:, :])
                nc.vector.tensor_add(nk2_sb[:, :], nk2_sb[:, :], nk2c_sb[:, :])
                nc.vector.tensor_add(g_sb[:, :], g_sb[:, :], gcorr_sb[:, :])

            # tnqinv[h*64+d] = temp_h / ||q_{h,d}||
            nqs_sb = small.tile([1, C], F32, tag="nqs")
            nc.scalar.activation(nqs_sb[:, :], nq2_sb[:, :],
                                 mybir.ActivationFunctionType.Sqrt)
            tnq_sb = small.tile([1, C], F32, tag="tnq")
            nc.vector.reciprocal(tnq_sb[:, :], nqs_sb[:, :])
            tnqf_sb = small.tile([1, C], F32, tag="tnqf")
            nc.vector.tensor_mul(tnqf_sb[:, :], tnq_sb[:, :], tempv_sb[:, :])
            # rearrange [1,(h,d)] -> [d, h] via DRAM round-trip
            scr = dram.tile([1, C], F32)
            nc.sync.dma_start(out=scr[:, :], in_=tnqf_sb[:, :])
            tnqT_sb = small.tile([HD, H], F32, tag="tnqT")
            nc.sync.dma_start(out=tnqT_sb[:, :],
                              in_=scr.rearrange("one (h d) -> (one d) h", d=HD))

            # nkinv broadcast to 64 partitions via K=1 matmul
            nks_sb = small.tile([1, C], F32, tag="nks")
            nc.scalar.activation(nks_sb[:, :], nk2_sb[:, :],
                                 mybir.ActivationFunctionType.Sqrt)
            nkinv_sb = small.tile([1, C], F32R, tag="nkinv")
            with nc.allow_low_precision(reason="fp32r rounding of 1/||k|| is fine"):
                nc.vector.reciprocal(nkinv_sb[:, :], nks_sb[:, :])
            nkbc_ps = []
            for f in range(2):
                b_ps = pss.tile([HD, FH], F32, tag="pss")
                nc.tensor.matmul(b_ps[:, :], ones1_sb[:, :],
                                 nkinv_sb[:, f * FH:(f + 1) * FH],
                                 start=True, stop=True)
                nkbc_ps.append(b_ps)

            # ---------------- phase 3: softmax + M build ----------------
            t1_sb = small.tile([HD, C], F32, tag="t1")
            for f in range(2):
                nc.vector.tensor_mul(t1_sb[:, f * FH:(f + 1) * FH],
                                     g_sb[:, f * FH:(f + 1) * FH], nkbc_ps[f][:, :])
            t2_sb = small.tile([HD, H, HD], F32, tag="t2")
            nc.vector.tensor_mul(
                t2_sb[:, :, :],
                t1_sb.rearrange("d (h e) -> d h e", h=H),
                tnqT_sb.unsqueeze(2).broadcast_to([HD, H, HD]))
            # |logits| <= max|temp| (Cauchy-Schwarz on normalized vectors):
            # safe to exp without max-subtraction for the given inputs.
            e_sb = small.tile([HD, H, HD], F32, tag="e")
            nc.scalar.activation(e_sb[:, :, :], t2_sb[:, :, :],
                                 mybir.ActivationFunctionType.Exp)
            sum_sb = small.tile([HD, H], F32, tag="sum")
            nc.vector.reduce_sum(sum_sb[:, :], e_sb[:, :, :], AX)
            rec_sb = small.tile([HD, H], F32, tag="rec")
            nc.vector.reciprocal(rec_sb[:, :], sum_sb[:, :])
            attn_sb = small.tile([HD, H, HD], F32R, tag="attn")
            nc.vector.tensor_mul(
                attn_sb[:, :, :], e_sb[:, :, :],
                rec_sb.unsqueeze(2).broadcast_to([HD, H, HD]))
            # parity-split attn so lhsT base partition matches Wproj rows:
            # attn2[(h%2)*64+d, h//2, e] = attn[d, h, e]
            attn2_sb = small.tile([128, H // 2, HD], F32R, tag="attn2")
            av = attn_sb.rearrange("d (j two) e -> d two j e", two=2)
            for p0 in range(2):
                nc.sync.dma_start(out=attn2_sb[p0 * HD:(p0 + 1) * HD, :, :],
                                  in_=av[:, p0, :, :])
            if debug:
                adbg_sb = small.tile([HD, C], F32, tag="adbg")
                nc.vector.tensor_copy(adbg_sb[:, :],
                                      attn_sb.bitcast(F32).rearrange("d h e -> d (h e)"))
                nc.sync.dma_start(out=adbg_d[:, :], in_=adbg_sb[:, :])
                nc.sync.dma_start(out=gdbg_d[:, :], in_=g_sb[:, :])

            # R_h = attn_h^T @ Wproj_h   [64(e), C]
            wproj_sb = big.tile([128, KC, C], F32R, tag="big")   # slot: wk's
            nc.sync.dma_start(out=wproj_sb[:, :, :],
                              in_=wproj_d.rearrange("(k p) c -> p k c", p=128))
            r_sb = big.tile([128, KC, C], F32R, tag="big")       # slot: sr's
            for h in range(H):
                po, pc = (h % 2) * HD, h // 2
                for f in range(2):
                    r_ps = pss.tile([HD, FH], F32, tag="pss")
                    nc.tensor.matmul(r_ps[:, :],
                                     attn2_sb[po:po + HD, pc, :],
                                     wproj_sb[po:po + HD, pc, f * FH:(f + 1) * FH],
                                     start=True, stop=True)
                    nc.vector.tensor_copy(r_sb[po:po + HD, pc, f * FH:(f + 1) * FH],
                                          r_ps[:, :])

            # M = sum_h Wv_h @ R_h   [C, C]
            wvt_sb = big.tile([128, KC, C], F32R, tag="big")     # slot: uq's
            nc.sync.dma_start(out=wvt_sb[:, :, :],
                              in_=wvt_d.rearrange("(k p) c -> p k c", p=128))
            m_sb = big.tile([128, KC, C], F32R, tag="big")       # slot: uk's
            # NB: matmuls within one PSUM accumulation group must keep the
            # same SBUF base partition (mixed 0/64 groups crash fp32r on HW),
            # so accumulate the two head parities separately and add on DVE.
            for m in range(KC):
                for f in range(2):
                    m_ps0 = ps.tile([128, FH], F32, tag="ps", name=f"m_ps0_{m}_{f}")
                    m_ps1 = ps.tile([128, FH], F32, tag="ps", name=f"m_ps1_{m}_{f}")
                    for j in range(H // 2):
                        nc.tensor.matmul(m_ps0[:, :],
                                         wvt_sb[0:HD, j, m * 128:(m + 1) * 128],
                                         r_sb[0:HD, j, f * FH:(f + 1) * FH],
                                         start=(j == 0), stop=(j == H // 2 - 1))
                    for j in range(H // 2):
                        nc.tensor.matmul(m_ps1[:, :],
                                         wvt_sb[HD:128, j, m * 128:(m + 1) * 128],
                                         r_sb[HD:128, j, f * FH:(f + 1) * FH],
                                         start=(j == 0), stop=(j == H // 2 - 1))
                    mtmp_sb = small.tile([128, FH], F32, tag="mtmp", bufs=2,
                                         name=f"mtmp_{m}_{f}")
                    nc.vector.tensor_copy(mtmp_sb[:, :], m_ps0[:, :])
                    nc.vector.tensor_tensor(m_sb[:, m, f * FH:(f + 1) * FH],
                                            mtmp_sb[:, :], m_ps1[:, :],
                                            mybir.AluOpType.add)
            if debug:
                mdbg_sb = yo.tile([128, C], F32, tag="y")
                for m in range(KC):
                    nc.vector.tensor_copy(mdbg_sb[:, :], m_sb.bitcast(F32)[:, m, :])
                    nc.sync.dma_start(out=mdbg_d[m * 128:(m + 1) * 128, :], in_=mdbg_sb[:, :])

            crow_sb = None
            ones128w_sb = None
            if has_bias:
                # c = sum_h bv_h^T R_h + bproj
                bvt_sb = const.tile([128, H // 2], F32R, tag="bvt")
                nc.sync.dma_start(out=bvt_sb[:, :], in_=bvt_d[:, :])
                bproj_sb = const.tile([1, C], F32, tag="bproj")
                nc.sync.dma_start(out=bproj_sb[:, :], in_=bproj_d[:, :])
                crow_sb = small.tile([1, C], F32R, tag="crow", bufs=1)
                for f in range(2):
                    c_ps0 = pss.tile([1, FH], F32, tag="pss", name=f"c_ps0_{f}")
                    c_ps1 = pss.tile([1, FH], F32, tag="pss", name=f"c_ps1_{f}")
                    for j in range(H // 2):
                        nc.tensor.matmul(c_ps0[:, :], bvt_sb[0:HD, j:j + 1],
                                         r_sb[0:HD, j, f * FH:(f + 1) * FH],
                                         start=(j == 0), stop=(j == H // 2 - 1))
                    for j in range(H // 2):
                        nc.tensor.matmul(c_ps1[:, :], bvt_sb[HD:128, j:j + 1],
                                         r_sb[HD:128, j, f * FH:(f + 1) * FH],
                                         start=(j == 0), stop=(j == H // 2 - 1))
                    tmpc_sb = small.tile([1, FH], F32, tag="tmpc", name=f"tmpc_{f}")
                    nc.vector.tensor_copy(tmpc_sb[:, :], c_ps0[:, :])
                    nc.vector.tensor_tensor(tmpc_sb[:, :], tmpc_sb[:, :], c_ps1[:, :],
                                            mybir.AluOpType.add)
                    nc.vector.tensor_tensor(crow_sb[:, f * FH:(f + 1) * FH],
                                            tmpc_sb[:, :], bproj_sb[:, f * FH:(f + 1) * FH],
                                            mybir.AluOpType.add)
                ones128w_sb = const.tile([1, 128], F32R, tag="ones128w")
                nc.sync.dma_start(out=ones128w_sb[:, :], in_=ones128w_d[:, :])

            # ---------------- phase 4: y = x @ M (+ c) ----------------
            TS = 256                       # token stream chunk
            xT_view = xT_d.rearrange("(k p) n -> p k n", p=128)
            for tb in range(NLOC // TS):
                xt_t = xts.tile([128, KC, TS], F32R, tag="xTs")
                nc.sync.dma_start(out=xt_t[:, :, :],
                                  in_=xT_view[:, :, tb * TS:(tb + 1) * TS])
                for sub in range(TS // 128):
                    y_sb = yo.tile([128, C], F32, tag="y")
                    for f in range(2):
                        y_ps = ps.tile([128, FH], F32, tag="ps")
                        for k in range(KC):
                            nc.tensor.matmul(y_ps[:, :],
                                             xt_t[:, k, sub * 128:(sub + 1) * 128],
                                             m_sb[:, k, f * FH:(f + 1) * FH],
                                             start=(k == 0),
                                             stop=(k == KC - 1 and not has_bias))
                        if has_bias:
                            nc.tensor.matmul(y_ps[:, :], ones128w_sb[:, :],
                                             crow_sb[:, f * FH:(f + 1) * FH],
                                             start=False, stop=True)
                        nc.vector.tensor_copy(y_sb[:, f * FH:(f + 1) * FH], y_ps[:, :])
                    row0 = tb * TS + sub * 128
                    nc.sync.dma_start(out=y_d[row0:row0 + 128, :], in_=y_sb[:, :])

    nc.compile()
    return nc


def _get_program(has_bias: bool, debug: bool = False):
    key = (has_bias, debug)
    if key not in _CACHE:
        _CACHE[key] = _build(has_bias, debug)
    return _CACHE[key]


def _prepare_inputs(x, Wqkv, bqkv, temperature, Wproj, bproj, has_bias):
    """Build the 8 per-core input maps (host-side sharding + fp32r prep)."""
    x = np.asarray(x, np.float32)
    Wqkv = np.asarray(Wqkv, np.float32)
    bqkv = np.asarray(bqkv, np.float32)
    temperature = np.asarray(temperature, np.float32)
    Wproj = np.asarray(Wproj, np.float32)
    bproj = np.asarray(bproj, np.float32)

    wq = _round_fp32r(Wqkv[:, :C])
    wk = _round_fp32r(Wqkv[:, C:2 * C])
    wvt = _round_fp32r(np.ascontiguousarray(Wqkv[:, 2 * C:].T))
    wproj = _round_fp32r(Wproj)
    tempv = np.repeat(temperature.reshape(H), HD).reshape(1, C).astype(np.float32)
    ones128 = np.ones((128, 1), np.float32)
    ones1 = np.ones((1, HD), np.float32)

    common = dict(wq=wq, wk=wk, wvt=wvt, wproj=wproj, tempv=tempv,
                  ones128=ones128, ones1=ones1)

    if has_bias:
        bq, bk, bv = bqkv[:C], bqkv[C:2 * C], bqkv[2 * C:]
        # colsum-dependent correction terms (one host pass over x)
        colsum = x.sum(axis=1, dtype=np.float64)            # [B, C]
        common["bvt"] = _round_fp32r(bv.reshape(H, HD).T.copy())
        common["bproj"] = bproj.reshape(1, C)
        common["ones128w"] = np.ones((1, 128), np.float32)

    in_maps = []
    for core in range(NCORES):
        b, half = core // 2, core % 2
        xl = _round_fp32r(x[b, half * NLOC:(half + 1) * NLOC, :])
        m = dict(common)
        m["x"] = xl
        m["xT"] = np.ascontiguousarray(xl.T)
        if has_bias:
            cs = colsum[b]                                   # [C]
            gc = np.zeros((HD, C), np.float32)
            nq2c = np.zeros((1, C), np.float32)
            nk2c = np.zeros((1, C), np.float32)
            for h in range(H):
                sl = slice(h * HD, (h + 1) * HD)
                csWk = cs @ Wqkv[:, C + h * HD:C + (h + 1) * HD].astype(np.float64)
                csWq = cs @ Wqkv[:, h * HD:(h + 1) * HD].astype(np.float64)
                gc[:, sl] = (np.outer(bq[sl], csWk) + np.outer(csWq, bk[sl])
                             + N * np.outer(bq[sl], bk[sl])).astype(np.float32)
                nq2c[0, sl] = (2 * bq[sl] * csWq + N * bq[sl] ** 2).astype(np.float32)
                nk2c[0, sl] = (2 * bk[sl] * csWk + N * bk[sl] ** 2).astype(np.float32)
            m["gcorr"] = gc
            m["nq2c"] = nq2c
            m["nk2c"] = nk2c
        in_maps.append(m)
    return in_maps


def kernel(x, Wqkv, bqkv, temperature, Wproj, bproj):
    from concourse import bass2jax
    has_bias = bool(np.any(np.asarray(bqkv)) or np.any(np.asarray(bproj)))
    nc = _get_program(has_bias)
    in_maps = _prepare_inputs(x, Wqkv, bqkv, temperature, Wproj, bproj, has_bias)
    results = bass2jax.run_bass_via_pjrt(nc, in_maps, n_cores=NCORES)
    out = np.empty((B, N, C), np.float32)
    for core in range(NCORES):
        b, half = core // 2, core % 2
        out[b, half * NLOC:(half + 1) * NLOC, :] = results[core]["y"]
    return out



# revision 20
# speedup vs baseline: 2.6255x; 2.6255x over previous
"""Cross-covariance (XCA / channel) attention kernel for Trainium2, 8 NeuronCores.

Reference computation (per batch b, head h, with X = x[b] in R^{N x C}):
    qkv = X @ Wqkv + bqkv;  q,k,v per head as [hd, N] (channels x tokens)
    q <- l2norm(q, axis=N) * temp_h ; k <- l2norm(k, axis=N)
    attn = softmax(q @ k^T)                # [hd, hd] channel attention
    out_h = attn @ v                       # [hd, N]
    y = concat_h(out_h)^T @ Wproj + bproj  # [N, C]

Key restructure (mathematically exact):
    All attention statistics derive from the per-batch Gram matrix
        S = X^T X   in R^{C x C}:
    G[h] = Wq_h^T S Wk_h          (q.k^T inner products, pre-normalization)
    ||q_d||^2 = diag(Wq_h^T S Wq_h),  ||k_e||^2 = diag(Wk_h^T S Wk_h)
    attn[h] = softmax(temp_h * G[h] / (||q|| ||k||^T))
    y = X @ M + c, where M = sum_h Wv_h @ attn[h]^T @ Wproj_h  in R^{C x C}

Sharding: 8 cores = 4 batches x 2 sequence halves. Each core computes the
FULL-batch S itself (no cross-core collective at all): S is symmetric, so
only the upper-triangle block-columns are accumulated (58% of the matmul
work) in a single pass over x, and the lower triangle is filled by 15 PE
transposes. Attention + M are tiny and computed redundantly per core; each
core then produces its own 4096 output rows y = x_half @ M.

Datapath is bf16 (inputs rounded on host) with fp32 PSUM accumulation:
halves HBM traffic and runs the PE at full rate for every free size.
The head dimension is laid out so head PAIRS share the 128 partitions,
letting R (block-diagonal attn) and M accumulate with K=128 contractions.
"""
import numpy as np
import ml_dtypes

import concourse.bacc as bacc
import concourse.mybir as mybir
import concourse.tile as tile

B, N, C = 4, 8192, 768
H, HD = 12, 64
NLOC = N // 2          # output tokens per core (4096)
NCORES = 8
F32 = mybir.dt.float32
F32R = mybir.dt.float32r
BF16 = mybir.dt.bfloat16
AX = mybir.AxisListType.X
BF = ml_dtypes.bfloat16

# upper-triangle block-column chunks of S: (block_row m, col_start, col_end)
TRI = [(0, 0, 512), (0, 512, 768),
       (1, 128, 640), (1, 640, 768),
       (2, 256, 768),
       (3, 384, 768),
       (4, 512, 768),
       (5, 640, 768)]

_CACHE = {}


def _build(has_bias: bool):
    nc = bacc.Bacc("TRN2", target_bir_lowering=False, debug=False,
                   enable_asserts=False, num_devices=NCORES)

    # ---- per-core I/O ----
    x_d = nc.dram_tensor("x", [N, C], BF16, kind="ExternalInput")       # full batch
    xT_d = nc.dram_tensor("xT", [C, NLOC], BF16, kind="ExternalInput")  # own half, T
    wq_d = nc.dram_tensor("wq", [C, C], BF16, kind="ExternalInput")
    wk_d = nc.dram_tensor("wk", [C, C], BF16, kind="ExternalInput")
    wvt_d = nc.dram_tensor("wvt", [C, C], BF16, kind="ExternalInput")
    wproj_d = nc.dram_tensor("wproj", [C, C], BF16, kind="ExternalInput")
    tempv_d = nc.dram_tensor("tempv", [1, C], F32, kind="ExternalInput")
    ones128_d = nc.dram_tensor("ones128", [128, 1], BF16, kind="ExternalInput")
    ones1_d = nc.dram_tensor("ones1", [1, HD], F32R, kind="ExternalInput")
    ident_d = nc.dram_tensor("ident", [128, 128], BF16, kind="ExternalInput")
    if has_bias:
        gcorr_d = nc.dram_tensor("gcorr", [HD, C], F32, kind="ExternalInput")
        nq2c_d = nc.dram_tensor("nq2c", [1, C], F32, kind="ExternalInput")
        nk2c_d = nc.dram_tensor("nk2c", [1, C], F32, kind="ExternalInput")
        bvt_d = nc.dram_tensor("bvt", [128, H // 2], BF16, kind="ExternalInput")
        bproj_d = nc.dram_tensor("bproj", [1, C], F32, kind="ExternalInput")
        ones128w_d = nc.dram_tensor("ones128w", [1, 128], BF16, kind="ExternalInput")
    y_d = nc.dram_tensor("y", [NLOC, C], F32, kind="ExternalOutput")

    NT = N // 128        # 64 token chunks of the full batch
    KC = C // 128        # 6 channel chunks
    KH = H // 2          # 6 head pairs
    FH = C // 2          # 384, free-dim half

    with tile.TileContext(nc) as tc:
        # GPSIMD/Pool cannot touch PSUM on HW: PSUM->SBUF copies alternate
        # between DVE (tensor_copy) and Act (Copy activation; 'copy' lives in
        # both the sqrt and exp table sets, so it never forces a table load)
        def pcopy(idx, out_ap, in_ap):
            if idx % 2 == 0:
                nc.vector.tensor_copy(out_ap, in_ap)
            else:
                nc.scalar.activation(out_ap, in_ap,
                                     mybir.ActivationFunctionType.Copy)

        with (
            tc.tile_pool(name="big", bufs=7) as big,       # [128,6,768] bf16 9KB slots
            tc.tile_pool(name="xs", bufs=8) as xs,         # x stream [128,768] bf16
            tc.tile_pool(name="xts", bufs=16) as xts,      # xT fully prefetched
            tc.tile_pool(name="pb", bufs=3) as pb,         # norm product blocks
            tc.tile_pool(name="yo", bufs=4) as yo,         # y out [128,768] f32
            tc.tile_pool(name="small", bufs=1) as small,   # small f32 tiles
            tc.tile_pool(name="const", bufs=1) as const,
            tc.tile_pool(name="pp", bufs=8, space="PSUM") as pp,  # 8 x 1-bank slots
            tc.tile_pool(name="dram", bufs=1, space="DRAM") as dram,
        ):
            # preload the sqrt/copy activation table under phase-1 compute
            dm_sb = small.tile([1, 8], F32, tag="dm")
            nc.vector.memset(dm_sb[:, :], 0.0)
            nc.scalar.activation(dm_sb[:, :], dm_sb[:, :],
                                 mybir.ActivationFunctionType.Sqrt)

            # ---------------- phase 1: S = x^T x (full batch, triangle) ----
            # weight/const tiles allocated up front; their DMAs are
            # interleaved into the token loop to stream under phase-1 compute
            wq_sb = big.tile([128, KC, C], BF16, tag="big")
            wk_sb = big.tile([128, KC, C], BF16, tag="big")
            wproj_sb = big.tile([128, KC, C], BF16, tag="big")
            wvt_sb = big.tile([128, KC, C], BF16, tag="big")

            s_ps = [pp.tile([128, c1 - c0], F32, tag="ps", name=f"s_ps{i}")
                    for i, (m, c0, c1) in enumerate(TRI)]
            x_view = x_d.rearrange("(t p) c -> p t c", p=128)
            for t in range(NT):
                x_t = xs.tile([128, C], BF16, tag="xt")
                nc.sync.dma_start(out=x_t[:, :], in_=x_view[:, t, :])
                for i, (m, c0, c1) in enumerate(TRI):
                    nc.tensor.matmul(s_ps[i][:, :],
                                     x_t[:, m * 128:(m + 1) * 128],
                                     x_t[:, c0:c1],
                                     start=(t == 0), stop=(t == NT - 1))
                if t == 8:
                    nc.sync.dma_start(out=wq_sb[:, :, :],
                                      in_=wq_d.rearrange("(k p) c -> p k c", p=128))
                elif t == 16:
                    nc.sync.dma_start(out=wk_sb[:, :, :],
                                      in_=wk_d.rearrange("(k p) c -> p k c", p=128))
                elif t == 24:
                    nc.sync.dma_start(out=wproj_sb[:, :, :],
                                      in_=wproj_d.rearrange("(k p) c -> p k c", p=128))
                elif t == 32:
                    nc.sync.dma_start(out=wvt_sb[:, :, :],
                                      in_=wvt_d.rearrange("(k p) c -> p k c", p=128))

            ident_sb = const.tile([128, 128], BF16, tag="ident")
            nc.sync.dma_start(out=ident_sb[:, :], in_=ident_d[:, :])
            ones128_sb = const.tile([128, 1], BF16, tag="ones128")
            nc.sync.dma_start(out=ones128_sb[:, :], in_=ones128_d[:, :])
            ones1_sb = const.tile([1, HD], F32R, tag="ones1")
            nc.sync.dma_start(out=ones1_sb[:, :], in_=ones1_d[:, :])
            tempv_sb = const.tile([1, C], F32, tag="tempv")
            nc.sync.dma_start(out=tempv_sb[:, :], in_=tempv_d[:, :])

            # assemble S in bf16: copy triangle chunks, transpose the rest
            sbf_sb = big.tile([128, KC, C], BF16, tag="big")
            with nc.allow_low_precision(reason="S in bf16 is within accuracy budget"):
                for i, (m, c0, c1) in enumerate(TRI):
                    pcopy(i, sbf_sb[:, m, c0:c1], s_ps[i][:, :])
                for idx, (i, j) in enumerate((i, j) for i in range(KC)
                                             for j in range(i + 1, KC)):
                    t_ps = pp.tile([128, 128], BF16, tag="ps", name=f"t_ps{i}_{j}")
                    nc.tensor.transpose(t_ps[:, :], sbf_sb[:, i, j * 128:(j + 1) * 128],
                                        ident_sb[:, :])
                    pcopy(idx + 1, sbf_sb[:, j, i * 128:(i + 1) * 128], t_ps[:, :])

            # ---------------- phase 2: U = S @ W with fused norm colsums ----
            # pass 0 (Wq): U_q only feeds nq2 = colsum(Wq * U_q) -> stays in PSUM
            # pass 1 (Wk): U_k kept in SBUF for G = blockdiag(Wq^T U_k)
            uk_sb = big.tile([128, KC, C], BF16, tag="big")
            n2_sbs = []
            for wi, w in enumerate((wq_sb, wk_sb)):
                n_ps = [pp.tile([1, FH], F32, tag="ps", name=f"n_ps{wi}{f}")
                        for f in range(2)]
                pend = None        # software-pipelined norm colsum (PE never waits)
                for m in range(KC):
                    for f in range(2):
                        u_ps = pp.tile([128, FH], F32, tag="ps")
                        for k in range(KC):
                            nc.tensor.matmul(u_ps[:, :],
                                             sbf_sb[:, k, m * 128:(m + 1) * 128],
                                             w[:, k, f * FH:(f + 1) * FH],
                                             start=(k == 0), stop=(k == KC - 1))
                        if pend is not None:
                            pm, pf, pp_t = pend
                            nc.tensor.matmul(n_ps[pf][:, :], ones128_sb[:, :],
                                             pp_t[:, :],
                                             start=(pm == 0), stop=(pm == KC - 1))
                        with nc.allow_low_precision(reason="bf16 norm products"):
                            if wi == 1:
                                pcopy(m + f, uk_sb[:, m, f * FH:(f + 1) * FH],
                                      u_ps[:, :])
                            p_t = pb.tile([128, FH], BF16, tag="pblk")
                            if wi == 1:
                                # Pool can mul from SBUF (never PSUM on HW)
                                nc.gpsimd.tensor_mul(
                                    p_t[:, :], w[:, m, f * FH:(f + 1) * FH],
                                    uk_sb[:, m, f * FH:(f + 1) * FH])
                            else:
                                nc.vector.tensor_mul(
                                    p_t[:, :], w[:, m, f * FH:(f + 1) * FH],
                                    u_ps[:, :])
                        pend = (m, f, p_t)
                pm, pf, pp_t = pend
                nc.tensor.matmul(n_ps[pf][:, :], ones128_sb[:, :], pp_t[:, :],
                                 start=(pm == 0), stop=(pm == KC - 1))
                n2_sb = small.tile([1, C], F32, tag="n2", bufs=2)
                for f in range(2):
                    nc.vector.tensor_copy(n2_sb[:, f * FH:(f + 1) * FH], n_ps[f][:, :])
                n2_sbs.append(n2_sb)
                if wi == 0:
                    # ||q|| chain runs on Act/DVE while the Wk pass uses the PE
                    nq2_sb = n2_sb
                    if has_bias:
                        nq2c_sb = const.tile([1, C], F32, tag="nq2c")
                        nc.sync.dma_start(out=nq2c_sb[:, :], in_=nq2c_d[:, :])
                        nc.vector.tensor_add(nq2_sb[:, :], nq2_sb[:, :], nq2c_sb[:, :])
                    nqs_sb = small.tile([1, C], F32, tag="nqs")
                    nc.scalar.activation(nqs_sb[:, :], nq2_sb[:, :],
                                         mybir.ActivationFunctionType.Sqrt)
                    tnq_sb = small.tile([1, C], F32, tag="tnq")
                    nc.vector.reciprocal(tnq_sb[:, :], nqs_sb[:, :])
                    tnqf_sb = small.tile([1, C], F32, tag="tnqf")
                    nc.vector.tensor_mul(tnqf_sb[:, :], tnq_sb[:, :], tempv_sb[:, :])
                    # rearrange [1,(h,d)] -> [d, h] via DRAM round-trip
                    scr = dram.tile([1, C], F32)
                    nc.sync.dma_start(out=scr[:, :], in_=tnqf_sb[:, :])
                    tnqT_sb = small.tile([HD, H], F32, tag="tnqT")
                    nc.sync.dma_start(out=tnqT_sb[:, :],
                                      in_=scr.rearrange("one (h d) -> (one d) h", d=HD))
            nk2_sb = n2_sbs[1]

            # G[h] = Wq_h^T (S Wk)_h  -> [64, (h,e)]
            g_sb = small.tile([HD, C], F32, tag="g", bufs=1)
            for h in range(H):
                g_ps = pp.tile([HD, HD], F32, tag="ps")
                for k in range(KC):
                    nc.tensor.matmul(g_ps[:, :],
                                     wq_sb[:, k, h * HD:(h + 1) * HD],
                                     uk_sb[:, k, h * HD:(h + 1) * HD],
                                     start=(k == 0), stop=(k == KC - 1))
                nc.vector.tensor_copy(g_sb[:, h * HD:(h + 1) * HD], g_ps[:, :])

            if has_bias:
                nk2c_sb = const.tile([1, C], F32, tag="nk2c")
                nc.sync.dma_start(out=nk2c_sb[:, :], in_=nk2c_d[:, :])
                gcorr_sb = const.tile([HD, C], F32, tag="gcorr")
                nc.sync.dma_start(out=gcorr_sb[:, :], in_=gcorr_d[:, :])
                nc.vector.tensor_add(nk2_sb[:, :], nk2_sb[:, :], nk2c_sb[:, :])
                nc.vector.tensor_add(g_sb[:, :], g_sb[:, :], gcorr_sb[:, :])

            # nkinv broadcast to 64 partitions via K=1 matmul
            nks_sb = small.tile([1, C], F32, tag="nks")
            nc.scalar.activation(nks_sb[:, :], nk2_sb[:, :],
                                 mybir.ActivationFunctionType.Sqrt)
            nkinv_sb = small.tile([1, C], F32R, tag="nkinv")
            with nc.allow_low_precision(reason="fp32r rounding of 1/||k|| is fine"):
                nc.vector.reciprocal(nkinv_sb[:, :], nks_sb[:, :])
            nkbc_ps = []
            for f in range(2):
                b_ps = pp.tile([HD, FH], F32, tag="ps", name=f"nkbc{f}")
                nc.tensor.matmul(b_ps[:, :], ones1_sb[:, :],
                                 nkinv_sb[:, f * FH:(f + 1) * FH],
                                 start=True, stop=True)
                nkbc_ps.append(b_ps)

            # ---------------- phase 3: softmax + M build ----------------
            # processed in two head-halves (heads 0-5 | 6-11) so the second
            # half's softmax chain overlaps the first half's R matmuls
            r_sb = big.tile([128, KC, C], BF16, tag="big")
            a2d_sbs = [small.tile([128, KH // 2, 128], BF16, tag=f"a2d{hf}",
                                  name=f"a2d{hf}")
                       for hf in range(2)]
            for a2d in a2d_sbs:
                nc.gpsimd.memset(a2d[:, :, :], 0.0)
            for hf in range(2):
                cols = slice(hf * FH, (hf + 1) * FH)       # heads 6hf..6hf+5
                t1_sb = small.tile([HD, FH], F32, tag="t1", bufs=2)
                nc.vector.tensor_mul(t1_sb[:, :], g_sb[:, cols], nkbc_ps[hf][:, :])
                t2_sb = small.tile([HD, KH, HD], F32, tag="t2", bufs=2)
                nc.vector.tensor_mul(
                    t2_sb[:, :, :],
                    t1_sb.rearrange("d (h e) -> d h e", h=KH),
                    tnqT_sb[:, hf * KH:(hf + 1) * KH].unsqueeze(2)
                    .broadcast_to([HD, KH, HD]))
                # |logits| <= max|temp| (Cauchy-Schwarz on normalized vectors):
                # safe to exp without max-subtraction for the given inputs.
                e_sb = small.tile([HD, KH, HD], F32, tag="e", bufs=2)
                nc.scalar.activation(e_sb[:, :, :], t2_sb[:, :, :],
                                     mybir.ActivationFunctionType.Exp)
                sum_sb = small.tile([HD, KH], F32, tag="sum", bufs=2)
                nc.vector.reduce_sum(sum_sb[:, :], e_sb[:, :, :], AX)
                rec_sb = small.tile([HD, KH], F32, tag="rec", bufs=2)
                nc.vector.reciprocal(rec_sb[:, :], sum_sb[:, :])
                attn_sb = small.tile([HD, KH, HD], BF16, tag="attn", bufs=2)
                with nc.allow_low_precision(reason="attn in bf16 is within budget"):
                    nc.vector.tensor_mul(
                        attn_sb[:, :, :], e_sb[:, :, :],
                        rec_sb.unsqueeze(2).broadcast_to([HD, KH, HD]))
                # block-diagonal parity layout so R uses K=128 contractions:
                # a2d[(h%2)*64+d, h//2, (h%2)*64+e] = attn[d, h, e], 0 elsewhere
                a2d = a2d_sbs[hf]
                av = attn_sb.rearrange("d (j two) e -> d two j e", two=2)
                for p0 in range(2):
                    nc.sync.dma_start(
                        out=a2d[p0 * HD:(p0 + 1) * HD, :, p0 * HD:(p0 + 1) * HD],
                        in_=av[:, p0, :, :])

                # R = blockdiag(attn)^T @ Wproj rows, head-pair at a time
                with nc.allow_low_precision(reason="R in bf16 is within budget"):
                    for jj in range(KH // 2):
                        j = hf * (KH // 2) + jj
                        for f in range(2):
                            r_ps = pp.tile([128, FH], F32, tag="ps")
                            nc.tensor.matmul(r_ps[:, :],
                                             a2d[:, jj, :],
                                             wproj_sb[:, j, f * FH:(f + 1) * FH],
                                             start=True, stop=True)
                            pcopy(j + f, r_sb[:, j, f * FH:(f + 1) * FH],
                                  r_ps[:, :])

            # M = sum_j Wv[:, pair j] @ R[pair j]   [C, C], K=128 per step
            m_sb = big.tile([128, KC, C], BF16, tag="big")
            with nc.allow_low_precision(reason="M in bf16 is within budget"):
                for m in range(KC):
                    for f in range(2):
                        m_ps = pp.tile([128, FH], F32, tag="ps")
                        for j in range(KH):
                            nc.tensor.matmul(m_ps[:, :],
                                             wvt_sb[:, j, m * 128:(m + 1) * 128],
                                             r_sb[:, j, f * FH:(f + 1) * FH],
                                             start=(j == 0), stop=(j == KH - 1))
                        pcopy(m + f, m_sb[:, m, f * FH:(f + 1) * FH], m_ps[:, :])

            crow_sb = None
            ones128w_sb = None
            if has_bias:
                # c = sum_h bv_h^T R_h + bproj
                bvt_sb = const.tile([128, KH], BF16, tag="bvt")
                nc.sync.dma_start(out=bvt_sb[:, :], in_=bvt_d[:, :])
                bproj_sb = const.tile([1, C], F32, tag="bproj")
                nc.sync.dma_start(out=bproj_sb[:, :], in_=bproj_d[:, :])
                crow_sb = small.tile([1, C], BF16, tag="crow", bufs=1)
                with nc.allow_low_precision(reason="bias row in bf16"):
                    for f in range(2):
                        c_ps = pp.tile([1, FH], F32, tag="ps", name=f"c_ps{f}")
                        for j in range(KH):
                            nc.tensor.matmul(c_ps[:, :], bvt_sb[:, j:j + 1],
                                             r_sb[:, j, f * FH:(f + 1) * FH],
                                             start=(j == 0), stop=(j == KH - 1))
                        nc.vector.tensor_tensor(crow_sb[:, f * FH:(f + 1) * FH],
                                                c_ps[:, :], bproj_sb[:, f * FH:(f + 1) * FH],
                                                mybir.AluOpType.add)
                ones128w_sb = const.tile([1, 128], BF16, tag="ones128w")
                nc.sync.dma_start(out=ones128w_sb[:, :], in_=ones128w_d[:, :])

            # ---------------- phase 4: y = x @ M (+ c), PSUM -> DRAM direct --
            TS = 256                       # token stream chunk
            xT_view = xT_d.rearrange("(k p) n -> p k n", p=128)
            for tb in range(NLOC // TS):
                xt_t = xts.tile([128, KC, TS], BF16, tag="xTs")
                nc.sync.dma_start(out=xt_t[:, :, :],
                                  in_=xT_view[:, :, tb * TS:(tb + 1) * TS])
                for sub in range(TS // 128):
                    row0 = tb * TS + sub * 128
                    y_sb = yo.tile([128, C], F32, tag="y")
                    for f in range(2):
                        y_ps = pp.tile([128, FH], F32, tag="ps")
                        for k in range(KC):
                            nc.tensor.matmul(y_ps[:, :],
                                             xt_t[:, k, sub * 128:(sub + 1) * 128],
                                             m_sb[:, k, f * FH:(f + 1) * FH],
                                             start=(k == 0),
                                             stop=(k == KC - 1 and not has_bias))
                        if has_bias:
                            nc.tensor.matmul(y_ps[:, :], ones128w_sb[:, :],
                                             crow_sb[:, f * FH:(f + 1) * FH],
                                             start=False, stop=True)
                        pcopy(f, y_sb[:, f * FH:(f + 1) * FH], y_ps[:, :])
                        # DMA each half as soon as its copy lands (shorter tail)
                        nc.sync.dma_start(out=y_d[row0:row0 + 128, f * FH:(f + 1) * FH],
                                          in_=y_sb[:, f * FH:(f + 1) * FH])

    nc.compile()
    return nc


def _get_program(has_bias: bool):
    if has_bias not in _CACHE:
        _CACHE[has_bias] = _build(has_bias)
    return _CACHE[has_bias]


def _prepare_inputs(x, Wqkv, bqkv, temperature, Wproj, bproj, has_bias):
    """Build the 8 per-core input maps (host-side sharding + bf16 prep)."""
    x = np.asarray(x, np.float32)
    Wqkv = np.asarray(Wqkv, np.float32)
    bqkv = np.asarray(bqkv, np.float32)
    temperature = np.asarray(temperature, np.float32)
    Wproj = np.asarray(Wproj, np.float32)
    bproj = np.asarray(bproj, np.float32)

    wq = np.asarray(Wqkv[:, :C], BF)
    wk = np.asarray(Wqkv[:, C:2 * C], BF)
    wvt = np.asarray(np.ascontiguousarray(Wqkv[:, 2 * C:].T), BF)
    wproj = np.asarray(Wproj, BF)
    tempv = np.repeat(temperature.reshape(H), HD).reshape(1, C).astype(np.float32)

    common = dict(wq=wq, wk=wk, wvt=wvt, wproj=wproj, tempv=tempv,
                  ones128=np.ones((128, 1), BF),
                  ones1=np.ones((1, HD), np.float32),
                  ident=np.eye(128, dtype=BF))

    if has_bias:
        bq, bk, bv = bqkv[:C], bqkv[C:2 * C], bqkv[2 * C:]
        colsum = x.sum(axis=1, dtype=np.float64)            # [B, C]
        common["bvt"] = np.asarray(bv.reshape(H // 2, 128).T, BF)
        common["bproj"] = bproj.reshape(1, C)
        common["ones128w"] = np.ones((1, 128), BF)

    xb = np.asarray(x, BF)                                  # [B, N, C] bf16
    in_maps = []
    for core in range(NCORES):
        b, half = core // 2, core % 2
        m = dict(common)
        m["x"] = xb[b]
        m["xT"] = np.ascontiguousarray(xb[b, half * NLOC:(half + 1) * NLOC, :].T)
        if has_bias:
            cs = colsum[b]                                   # [C]
            gc = np.zeros((HD, C), np.float32)
            nq2c = np.zeros((1, C), np.float32)
            nk2c = np.zeros((1, C), np.float32)
            for h in range(H):
                sl = slice(h * HD, (h + 1) * HD)
                csWk = cs @ Wqkv[:, C + h * HD:C + (h + 1) * HD].astype(np.float64)
                csWq = cs @ Wqkv[:, h * HD:(h + 1) * HD].astype(np.float64)
                gc[:, sl] = (np.outer(bq[sl], csWk) + np.outer(csWq, bk[sl])
                             + N * np.outer(bq[sl], bk[sl])).astype(np.float32)
                nq2c[0, sl] = (2 * bq[sl] * csWq + N * bq[sl] ** 2).astype(np.float32)
                nk2c[0, sl] = (2 * bk[sl] * csWk + N * bk[sl] ** 2).astype(np.float32)
            m["gcorr"] = gc
            m["nq2c"] = nq2c
            m["nk2c"] = nk2c
        in_maps.append(m)
    return in_maps


def kernel(x, Wqkv, bqkv, temperature, Wproj, bproj):
    from concourse import bass2jax
    has_bias = bool(np.any(np.asarray(bqkv)) or np.any(np.asarray(bproj)))
    nc = _get_program(has_bias)
    in_maps = _prepare_inputs(x, Wqkv, bqkv, temperature, Wproj, bproj, has_bias)
    results = bass2jax.run_bass_via_pjrt(nc, in_maps, n_cores=NCORES)
    out = np.empty((B, N, C), np.float32)
    for core in range(NCORES):
        b, half = core // 2, core % 2
        out[b, half * NLOC:(half + 1) * NLOC, :] = results[core]["y"]
    return out
